# revision 12
# baseline (speedup 1.0000x reference)
"""Trainium2 Bass kernel for nn_Attention_39608188404100.

Windowed-attention block (ViT-style, N=197 tokens) with SSF affines, relative
position bias, DCF head mixing, and output projection.

Strategy: pure data-parallel over batch across 8 NeuronCores (B=64 -> 8/core).
All weights are replicated; no collectives. Compute in bf16 on the
TensorEngine (fp32 PSUM accumulation).

Per core (BL=8 batches): each batch's 197 tokens are padded to 200 positions
and PERMUTED on host: position p = c*100 + ml*10 + g holds token
m = c*100 + g*10 + ml (c = chunk, 2x100). The 3 dummy positions per batch get
zero x-columns and a -40 relative-bias on their key rows, so exp() kills them
in the softmax; dummy query columns are dropped on host after download.

v3 structure (keeps the TensorEngine continuously fed):
  - The QKV projection for q/k is interleaved with attention phase A: the
    12 output-channel tiles are produced in 3 "ticks" of 4 (the q and k
    tiles of 4 heads), and immediately after each tick all 8 batches run
    scores -> exp -> *relb -> denominators -> normalize for those 4 heads
    on small [100,1600] tiles, which are dumped incrementally to DRAM
    scratch (scrA). qk lives in 12 separate tiles so the dependency
    tracker serializes only against the right tick.
  - Softmax denominator: ones[100,100]-stationary matmul replicates the
    column sums across partitions; reciprocal_approx_fast (fp32) runs
    partition-parallel.
  - The v projection runs after the t-loop (overlapping the first
    attention-B DMA gathers), then xT is freed.
  - Phase B per batch: gather to (wgi,h)-partition layout (hop2, 800B
    runs), block-diagonal [120x120] DCF mix matmul (drained on Vector),
    scatter (hop3) + contiguous gather (hop4) to key-partition layout,
    AV, projection (bias added in the Vector drain), bf16 output store.
  - DRAM scratch byte orders match the SBUF side of each write hop, so
    every shuffle DMA is <=3 AP dims with 800B-1.6KB contiguous runs.

Env:
  BASS_KERNEL_PROFILE=1  capture neuron-profile (exec_time_ns) on the run.
"""
import os
import sys

sys.path.insert(0, "/opt/trn_rl_repo")

import numpy as np
import ml_dtypes

import concourse.bass as bass
import concourse.tile as tile
from concourse import bacc, mybir

BF16 = mybir.dt.bfloat16
F32 = mybir.dt.float32
AF = mybir.ActivationFunctionType
ALU = mybir.AluOpType

B, N, C, H, DH = 64, 197, 768, 12, 64
NCORES = 8
BL = B // NCORES          # 8 batches per core
P2 = 200                  # padded positions per batch
T2 = BL * P2              # 1600 positions per core
SCALE = DH ** -0.5
KT = 6                    # contraction tiles of 128 over C=768
QKM = 12                  # 128-wide M tiles over 1536 q/k channels
TOK_CHUNKS = [(0, 512), (512, 512), (1024, 512), (1536, 64)]
DUMMY_BIAS = -40.0

_COMPILED = {}


def _build_graph():
    # detect_race_conditions=False: the sim race-detector's shadow model
    # linearizes multi-dim DMA APs (the mix shuffle) as byte offsets and
    # reports false overlaps between distinct pool slots; the value
    # semantics were validated against hardware.
    nc = bacc.Bacc(
        "TRN2", target_bir_lowering=False, debug=False,
        detect_race_conditions=False,
    )

    xT_d = nc.dram_tensor("xT", [128, KT * T2], BF16, kind="ExternalInput")
    wqk_d = nc.dram_tensor("wqk", [128, KT * 1536], BF16, kind="ExternalInput")
    wv_d = nc.dram_tensor("wv", [128, KT * 768], BF16, kind="ExternalInput")
    wp_d = nc.dram_tensor("wp", [128, KT * 768], BF16, kind="ExternalInput")
    relb_d = nc.dram_tensor("relb", [100, 2 * H * P2], BF16, kind="ExternalInput")
    mix_d = nc.dram_tensor("mixblk", [120, 120], BF16, kind="ExternalInput")
    bqk_d = nc.dram_tensor("bqk", [128, QKM], F32, kind="ExternalInput")
    bv_d = nc.dram_tensor("bv", [128, 768], BF16, kind="ExternalInput")
    bp_d = nc.dram_tensor("bp", [128, 768], BF16, kind="ExternalInput")
    out_d = nc.dram_tensor("out", [T2, 768], BF16, kind="ExternalOutput")

    with tile.TileContext(nc) as tc:
        with (
            tc.tile_pool(name="const", bufs=1) as cpool,
            tc.tile_pool(name="qkv", bufs=1) as qkvpool,
            tc.tile_pool(name="mxin", bufs=3) as mxpool,
            tc.tile_pool(name="mxout", bufs=2) as mopool,
            tc.tile_pool(name="dram", bufs=2, space=bass.MemorySpace.DRAM) as drpool,
            tc.tile_pool(name="dramA", bufs=BL, space=bass.MemorySpace.DRAM) as drApool,
            tc.tile_pool(name="psA", bufs=2, space=bass.MemorySpace.PSUM) as psA,
            tc.tile_pool(name="psM", bufs=2, space=bass.MemorySpace.PSUM) as psM,
        ):
            # ---- constants ----
            # wqk/xT live in per-kt tiles so the first GEMM only waits on
            # the first sixth of the input load.
            wqk = [cpool.tile([128, 1536], BF16, name=f"wqk{kt}")
                   for kt in range(KT)]
            wv = cpool.tile([128, KT * 768], BF16)
            wp = cpool.tile([128, KT * 768], BF16)
            relb = cpool.tile([100, 2 * H * P2], BF16)
            mixblk = cpool.tile([120, 120], BF16)
            bqk = cpool.tile([128, QKM], F32)
            bv = cpool.tile([128, 768], BF16)   # v-bias rows, replicated
            bp = cpool.tile([128, 768], BF16)   # proj-bias rows, replicated
            ones_den = cpool.tile([128, 128], BF16)
            nc.vector.memset(ones_den[:], 1.0)

            # persistent per-core activations. qk lives in 12 separate
            # tiles (one per 128-channel group) so attention ticks only
            # depend on their own stage-1 GEMMs.
            qk_t = [qkvpool.tile([128, T2], BF16, name=f"qk{mt}")
                    for mt in range(QKM)]
            v_sb = qkvpool.tile([100, 2 * BL * 768], BF16)
            scrA = [drApool.tile([10, 10, H, 2 * P2], BF16, tag="scrA",
                                 name=f"scrA{b}")
                    for b in range(BL)]

            with (
                tc.tile_pool(name="xin", bufs=1) as xpool,
                tc.tile_pool(name="et", bufs=4) as etpool,
                tc.tile_pool(name="dch", bufs=3) as dchpool,
                tc.tile_pool(name="psS", bufs=3, space=bass.MemorySpace.PSUM) as psS,
                tc.tile_pool(name="psD", bufs=1, space=bass.MemorySpace.PSUM) as psD,
            ):
                xk = [xpool.tile([128, T2], BF16, name=f"xk{kt}")
                      for kt in range(KT)]
                for kt in range(KT):
                    nc.sync.dma_start(
                        xk[kt][:], xT_d[:, kt * T2 : (kt + 1) * T2]
                    )
                    nc.sync.dma_start(
                        wqk[kt][:], wqk_d[:, kt * 1536 : (kt + 1) * 1536]
                    )
                nc.sync.dma_start(relb[:], relb_d[:])
                nc.sync.dma_start(bqk[:], bqk_d[:])
                nc.sync.dma_start(wv[:], wv_d[:])
                nc.sync.dma_start(bv[:], bv_d[:])
                nc.sync.dma_start(mixblk[:], mix_d[:])
                nc.sync.dma_start(wp[:], wp_d[:])
                nc.sync.dma_start(bp[:], bp_d[:])

                def qk_gemm(mt):
                    for (n0, nsz) in TOK_CHUNKS:
                        ps = psA.tile([128, 512], F32, tag="a")
                        for kt in range(KT):
                            nc.tensor.matmul(
                                ps[:, 0:nsz],
                                wqk[kt][:, mt * 128 : (mt + 1) * 128],
                                xk[kt][:, n0 : n0 + nsz],
                                start=(kt == 0),
                                stop=(kt == KT - 1),
                            )
                        nc.scalar.activation(
                            qk_t[mt][:, n0 : n0 + nsz],
                            ps[:, 0:nsz],
                            AF.Identity,
                            bias=bqk[:, mt : mt + 1],
                            scale=1.0,
                        )

                def a_chunk(b, tq):
                    """scores/exp/relb/den/normalize for heads 4tq..4tq+3 of
                    batch b, dumped to scrA[b][:, :, 4tq:4tq+4, :]."""
                    et = etpool.tile([100, 4 * 2 * P2], BF16, tag="et")
                    etv = et[:].rearrange("p (h c n) -> p h c n", h=4, c=2, n=P2)
                    for hh in range(4):
                        h = 4 * tq + hh
                        prow = (h % 2) * 64
                        qt = qk_t[h // 2]
                        kt_ = qk_t[6 + h // 2]
                        ps1 = psS.tile([128, 512], F32, tag="s")
                        nc.tensor.matmul(
                            ps1[0:100, 0:P2],
                            kt_[prow : prow + 64, b * P2 : b * P2 + 100],
                            qt[prow : prow + 64, b * P2 : b * P2 + P2],
                            start=True, stop=True,
                        )
                        nc.tensor.matmul(
                            ps1[0:100, P2 : 2 * P2],
                            kt_[prow : prow + 64, b * P2 + 100 : b * P2 + 200],
                            qt[prow : prow + 64, b * P2 : b * P2 + P2],
                            start=True, stop=True,
                        )
                        nc.scalar.activation(
                            et[0:100, hh * 2 * P2 : (hh + 1) * 2 * P2],
                            ps1[0:100, 0 : 2 * P2], AF.Exp,
                        )
                    # numerator: exp(scores) * exp(bias)
                    nc.vector.tensor_tensor(
                        et[0:100, :], et[0:100, :],
                        relb[0:100, 4 * tq * 2 * P2 : (4 * tq + 4) * 2 * P2],
                        ALU.mult,
                    )
                    # denominators (replicated across partitions by the PE)
                    # and in-place normalize
                    dch = dchpool.tile([100, 800], F32, tag="dch")
                    for dd in range(2):
                        psd = psD.tile([128, 512], F32, tag="d")
                        nc.tensor.matmul(
                            psd[0:100, 0:400],
                            ones_den[0:100, 0:100],
                            etv[:, 2 * dd : 2 * dd + 2, 0, :],
                            start=True, stop=False,
                        )
                        nc.tensor.matmul(
                            psd[0:100, 0:400],
                            ones_den[0:100, 0:100],
                            etv[:, 2 * dd : 2 * dd + 2, 1, :],
                            start=False, stop=True,
                        )
                        nc.vector.reciprocal_approx_fast(
                            dch[:, dd * 400 : (dd + 1) * 400],
                            psd[0:100, 0:400],
                        )
                    dv4 = dch[:].rearrange("p (h n) -> p h n", h=4)
                    for c in range(2):
                        nc.vector.tensor_tensor(
                            etv[:, :, c, :], etv[:, :, c, :], dv4, ALU.mult
                        )
                    nc.sync.dma_start(
                        scrA[b][:].rearrange(
                            "j wgi h cn -> (j wgi) h cn"
                        )[:, 4 * tq : 4 * tq + 4, :],
                        et[0:100, :].rearrange("p (h cn) -> p h cn", cn=2 * P2),
                    )

                # ---- interleaved stage 1 + attention phase A ----
                for tq in range(3):
                    for mt in (2 * tq, 2 * tq + 1, 6 + 2 * tq, 6 + 2 * tq + 1):
                        qk_gemm(mt)
                    for b in range(BL):
                        a_chunk(b, tq)

                # early B-phase gathers overlap the v projection below
                def hop2(b):
                    mxin = mxpool.tile([120, 10 * 2 * P2], BF16, tag="mxin")
                    nc.sync.dma_start(
                        mxin[:].rearrange("r (j cn) -> r j cn", j=10),
                        scrA[b][:].rearrange("j wgi h cn -> (wgi h) j cn"),
                    )
                    return mxin

                mx0 = hop2(0)
                mx1 = hop2(1)

                # ---- v = xT.T @ Wv', bias added in the Vector drain ----
                for b in range(BL):
                    for c in range(2):
                        base = b * P2 + c * 100
                        for (n0, nsz) in [(0, 512), (512, 256)]:
                            ps = psA.tile([128, 512], F32, tag="a")
                            for kt in range(KT):
                                nc.tensor.matmul(
                                    ps[0:100, 0:nsz],
                                    xk[kt][:, base : base + 100],
                                    wv[:, kt * 768 + n0 : kt * 768 + n0 + nsz],
                                    start=(kt == 0),
                                    stop=(kt == KT - 1),
                                )
                            nc.vector.tensor_tensor(
                                v_sb[0:100, (b * 2 + c) * 768 + n0 : (b * 2 + c) * 768 + n0 + nsz],
                                ps[0:100, 0:nsz],
                                bv[0:100, n0 : n0 + nsz],
                                ALU.add,
                            )

            # xT/et/dch + psS freed; phase-B SBUF pools reuse the space.
            with (
                tc.tile_pool(name="a2", bufs=2) as a2pool,
                tc.tile_pool(name="ao", bufs=2) as aopool,
                tc.tile_pool(name="osb", bufs=2) as opool,
                tc.tile_pool(name="psV", bufs=2, space=bass.MemorySpace.PSUM) as psV,
            ):
                def b_mix(b, mxin):
                    """mix matmul -> shuffle back (hop3 + hop4)."""
                    mxo = mopool.tile([120, 10 * 2 * P2], BF16, tag="mxo")
                    for i, o in enumerate(range(0, 4000, 500)):
                        psm = psM.tile([128, 512], F32, tag="m")
                        nc.tensor.matmul(
                            psm[0:120, 0:500], mixblk[:],
                            mxin[:, o : o + 500],
                            start=True, stop=True,
                        )
                        nc.vector.tensor_scalar_add(
                            mxo[:, o : o + 500], psm[0:120, 0:500], 0.0
                        )

                    # scrB byte order (j, wgi, k, cn): hop3 scatters 800B
                    # runs; hop4 is a fully contiguous gather into a2.
                    scrB = drpool.tile([10, 10, H, 2 * P2], BF16, tag="scrB")
                    nc.sync.dma_start(
                        scrB[:].rearrange("j wgi k cn -> (wgi k) j cn"),
                        mxo[:].rearrange("r (j cn) -> r j cn", j=10),
                    )
                    a2 = a2pool.tile([100, 2 * H * P2], BF16, tag="a2")
                    nc.sync.dma_start(
                        a2[0:100, :],
                        scrB[:].rearrange("j wgi k cn -> (j wgi) (k cn)"),
                    )
                    return a2

                def b_avproj(b, a2):
                    """AV (head pairs share a psum via tile_position) and
                    the output projection (bias added in the Vector drain)."""
                    aoT = aopool.tile([128, KT * P2], BF16, tag="ao")
                    for jj in range(H // 2):
                        pv = psV.tile([128, 512], F32, tag="v")
                        for sub in range(2):
                            k = 2 * jj + sub
                            rows = pv[sub * 64 : sub * 64 + 64, 0:P2]
                            tp = (0, sub * 64)
                            for c in range(2):
                                nc.tensor.matmul(
                                    rows,
                                    v_sb[0:100, (b * 2 + c) * 768 + k * 64 : (b * 2 + c) * 768 + (k + 1) * 64],
                                    a2[0:100, k * 2 * P2 + c * P2 : k * 2 * P2 + c * P2 + P2],
                                    start=(c == 0),
                                    stop=(c == 1),
                                    tile_position=tp,
                                )
                        nc.scalar.copy(
                            aoT[:, jj * P2 : (jj + 1) * P2], pv[:, 0:P2]
                        )

                    for (t0, tsz) in [(0, 128), (128, 72)]:
                        osb = opool.tile([128, 768], BF16, tag="osb")
                        for (n0, nsz) in [(0, 512), (512, 256)]:
                            pp = psA.tile([128, 512], F32, tag="a")
                            for kt in range(KT):
                                nc.tensor.matmul(
                                    pp[0:tsz, 0:nsz],
                                    aoT[:, kt * P2 + t0 : kt * P2 + t0 + tsz],
                                    wp[:, kt * 768 + n0 : kt * 768 + n0 + nsz],
                                    start=(kt == 0),
                                    stop=(kt == KT - 1),
                                )
                            nc.vector.tensor_tensor(
                                osb[0:tsz, n0 : n0 + nsz],
                                pp[0:tsz, 0:nsz],
                                bp[0:tsz, n0 : n0 + nsz],
                                ALU.add,
                            )
                        nc.sync.dma_start(
                            out_d[b * P2 + t0 : b * P2 + t0 + tsz, :], osb[0:tsz, :]
                        )

                # software pipeline: mix/shuffle of batch b+1 is emitted
                # ahead of AV/projection of batch b.
                mx = {0: mx0, 1: mx1}
                mx[2] = hop2(2)
                a2s = {0: b_mix(0, mx.pop(0))}
                for b in range(BL):
                    if b + 3 < BL:
                        mx[b + 3] = hop2(b + 3)
                    if b + 1 < BL:
                        a2s[b + 1] = b_mix(b + 1, mx.pop(b + 1))
                    b_avproj(b, a2s.pop(b))

    nc.compile()
    return nc


def _tile6(a, width):
    """[768, M] -> [128, 6*M] (K-tile-major host layout)."""
    assert a.shape == (768, width)
    return np.ascontiguousarray(
        a.reshape(KT, 128, width).transpose(1, 0, 2).reshape(128, KT * width)
    )


def _to_bf16(a):
    return np.asarray(a, dtype=np.float32).astype(ml_dtypes.bfloat16)


def _posmaps():
    """token m -> padded position p, and p -> m (or -1 for dummies)."""
    pos_of_tok = np.empty(N, np.int64)
    for m in range(N):
        c = 0 if m < 100 else 1
        mm = m - c * 100
        g, ml = mm // 10, mm % 10
        pos_of_tok[m] = c * 100 + ml * 10 + g
    tok_of_pos = np.full(P2, -1, np.int64)
    tok_of_pos[pos_of_tok] = np.arange(N)
    return pos_of_tok, tok_of_pos


_POS_OF_TOK, _TOK_OF_POS = _posmaps()


def _preprocess(inputs):
    x = np.asarray(inputs["x"], np.float32)
    qkv_w = np.asarray(inputs["qkv_w"], np.float32)
    q_bias = np.asarray(inputs["q_bias"], np.float32)
    v_bias = np.asarray(inputs["v_bias"], np.float32)
    sq = np.asarray(inputs["ssf_scale_qkv"], np.float32)
    tq = np.asarray(inputs["ssf_shift_qkv"], np.float32)
    rbt = np.asarray(inputs["rel_bias_table"], np.float32)
    coeff = np.asarray(inputs["bases_coeff"], np.float32)
    proj_w = np.asarray(inputs["proj_w"], np.float32)
    proj_b = np.asarray(inputs["proj_b"], np.float32)
    sp = np.asarray(inputs["ssf_scale_proj"], np.float32)
    tp = np.asarray(inputs["ssf_shift_proj"], np.float32)
    rel_index = np.asarray(inputs["rel_index"], np.int64)

    qkv_bias = np.concatenate([q_bias, np.zeros_like(q_bias), v_bias])
    w_eff = (qkv_w * sq[:, None]).copy()
    b_eff = (qkv_bias * sq + tq).copy()
    w_eff[0:768] *= SCALE
    b_eff[0:768] *= SCALE

    wqk = _tile6(np.ascontiguousarray(w_eff[0:1536].T), 1536)
    wvt = _tile6(np.ascontiguousarray(w_eff[1536:].T), 768)
    wp_eff = proj_w * sp[:, None]
    bp_eff = proj_b * sp + tp
    wpt = _tile6(np.ascontiguousarray(wp_eff.T), 768)

    bqk_sb = np.ascontiguousarray(b_eff[0:1536].reshape(QKM, 128).T).astype(np.float32)

    # rel bias in permuted+padded coordinates:
    # relb[p, (h*2+c)*P2 + n] = table[rel_index[qtok(n), ktok(c,p)], h]
    # dummy keys get DUMMY_BIAS, dummy queries 0.
    gathered = rbt[rel_index]                      # [query-tok, key-tok, H]
    relb4 = np.zeros((100, H, 2, P2), np.float32)
    q_valid = _TOK_OF_POS >= 0                     # [P2]
    qtok = np.where(q_valid, _TOK_OF_POS, 0)
    for c in range(2):
        ktok_pos = _TOK_OF_POS[c * 100 : (c + 1) * 100]   # [100]
        k_valid = ktok_pos >= 0
        ktok = np.where(k_valid, ktok_pos, 0)
        # blk[p, h, n] = gathered[qtok[n], ktok[p], h]
        blk = gathered[qtok[None, :], ktok[:, None], :]   # [100, P2, H]
        blk = blk.transpose(0, 2, 1)                      # [100, H, P2]
        blk = np.where(q_valid[None, None, :], blk, 0.0)
        blk = np.where(k_valid[:, None, None], blk, DUMMY_BIAS)
        relb4[:, :, c, :] = blk
    # upload exp(bias): the kernel multiplies exp(scores) by this instead
    # of adding the bias before the exp (dummy keys -> exp(-40) ~ 0).
    relb = np.exp(relb4.reshape(100, 2 * H * P2))

    # mix = coeff^T * 1.0 + I ; mixblk[wgi*12+h, wgi'*12+k] = d(wgi,wgi')mix[h,k]
    mix = coeff.T + np.eye(H, dtype=np.float32)
    mixblk = np.kron(np.eye(10, dtype=np.float32), mix)
    bv_rep = np.broadcast_to(b_eff[1536:].reshape(1, 768), (128, 768))
    bp_rep = np.broadcast_to(bp_eff.reshape(1, 768), (128, 768))

    common = {
        "wqk": _to_bf16(wqk),
        "wv": _to_bf16(wvt),
        "wp": _to_bf16(wpt),
        "relb": _to_bf16(relb),
        "mixblk": _to_bf16(mixblk),
        "bqk": bqk_sb,
        "bv": _to_bf16(bv_rep),
        "bp": _to_bf16(bp_rep),
    }
    in_maps = []
    for ci in range(NCORES):
        xs = x[ci * BL : (ci + 1) * BL]             # [BL, N, C]
        xp = np.zeros((BL, P2, C), np.float32)
        xp[:, _POS_OF_TOK, :] = xs
        xt = xp.reshape(BL * P2, C).T               # [C, T2]
        m = dict(common)
        m["xT"] = _to_bf16(_tile6(np.ascontiguousarray(xt), T2))
        in_maps.append(m)
    return in_maps


def _get_compiled():
    if "nc" not in _COMPILED:
        _COMPILED["nc"] = _build_graph()
    return _COMPILED["nc"]


LAST_EXEC_NS = None
LAST_RESULTS = None


def _ensure_ntff_hook():
    """The agent image's antenv package lacks axon_hooks; synthesize it so
    run_bass_kernel_spmd(trace=True) can capture NTFF profiles."""
    import types

    if "antenv.axon_hooks" in sys.modules:
        return
    try:
        sys.path.insert(0, "/root/.axon_site")
        from trn_agent_boot.trn_boot import _ntff_profile_via_ctypes

        hook = _ntff_profile_via_ctypes("/opt/axon/libaxon_pjrt.so")
    except Exception:
        hook = None
    mod = types.ModuleType("antenv.axon_hooks")
    _state = {"hook": hook}
    mod.get_axon_ntff_profile_hook = lambda: _state["hook"]
    mod.set_axon_ntff_profile_hook = lambda h: _state.__setitem__("hook", h)
    sys.modules["antenv.axon_hooks"] = mod


def kernel(**inputs) -> np.ndarray:
    global LAST_EXEC_NS, LAST_RESULTS
    nc = _get_compiled()
    in_maps = _preprocess(inputs)
    from concourse.bass_utils import run_bass_kernel_spmd

    trace = os.environ.get("BASS_KERNEL_PROFILE", "0") == "1"
    if trace:
        _ensure_ntff_hook()
    res = run_bass_kernel_spmd(nc, in_maps, core_ids=list(range(NCORES)), trace=trace)
    LAST_EXEC_NS = res.exec_time_ns
    LAST_RESULTS = res
    outs = []
    for i in range(NCORES):
        o = np.asarray(res.results[i]["out"], dtype=np.float32).reshape(BL, P2, C)
        outs.append(o[:, _POS_OF_TOK, :])           # drop dummies, un-permute
    return np.concatenate(outs, axis=0).astype(np.float32)


# revision 13
# speedup vs baseline: 1.1162x; 1.1162x over previous
"""Trainium2 Bass kernel for nn_Attention_39608188404100.

Windowed-attention block (ViT-style, N=197 tokens) with SSF affines, relative
position bias, DCF head mixing, and output projection.

Strategy: pure data-parallel over batch across 8 NeuronCores (B=64 -> 8/core).
All weights are replicated; no collectives. Compute in bf16 on the
TensorEngine (fp32 PSUM accumulation).

Per core (BL=8 batches): each batch's 197 tokens are padded to 200 positions
and PERMUTED on host: position p = c*100 + ml*10 + g holds token
m = c*100 + g*10 + ml (c = chunk, 2x100). The 3 dummy positions per batch get
zero x-columns and a -40 relative-bias on their key rows, so exp() kills them
in the softmax; dummy query columns are dropped on host after download.

v3 structure (keeps the TensorEngine continuously fed):
  - The QKV projection for q/k is interleaved with attention phase A: the
    12 output-channel tiles are produced in 3 "ticks" of 4 (the q and k
    tiles of 4 heads), and immediately after each tick all 8 batches run
    scores -> exp -> *relb -> denominators -> normalize for those 4 heads
    on small [100,1600] tiles, which are dumped incrementally to DRAM
    scratch (scrA). qk lives in 12 separate tiles so the dependency
    tracker serializes only against the right tick.
  - Softmax denominator: ones[100,100]-stationary matmul replicates the
    column sums across partitions; reciprocal_approx_fast (fp32) runs
    partition-parallel.
  - The v projection runs after the t-loop (overlapping the first
    attention-B DMA gathers), then xT is freed.
  - Phase B per batch: gather to (wgi,h)-partition layout (hop2, 800B
    runs), block-diagonal [120x120] DCF mix matmul (drained on Vector),
    scatter (hop3) + contiguous gather (hop4) to key-partition layout,
    AV, projection (bias added in the Vector drain), bf16 output store.
  - DRAM scratch byte orders match the SBUF side of each write hop, so
    every shuffle DMA is <=3 AP dims with 800B-1.6KB contiguous runs.

Env:
  BASS_KERNEL_PROFILE=1  capture neuron-profile (exec_time_ns) on the run.
"""
import os
import sys

sys.path.insert(0, "/opt/trn_rl_repo")

import numpy as np
import ml_dtypes

import concourse.bass as bass
import concourse.tile as tile
from concourse import bacc, mybir

BF16 = mybir.dt.bfloat16
F32 = mybir.dt.float32
AF = mybir.ActivationFunctionType
ALU = mybir.AluOpType

B, N, C, H, DH = 64, 197, 768, 12, 64
NCORES = 8
BL = B // NCORES          # 8 batches per core
P2 = 200                  # padded positions per batch
T2 = BL * P2              # 1600 positions per core
SCALE = DH ** -0.5
KT = 6                    # contraction tiles of 128 over C=768
QKM = 12                  # 128-wide M tiles over 1536 q/k channels
TOK_CHUNKS = [(0, 512), (512, 512), (1024, 512), (1536, 64)]
DUMMY_BIAS = -40.0

_COMPILED = {}


def _build_graph():
    # detect_race_conditions=False: the sim race-detector's shadow model
    # linearizes multi-dim DMA APs (the mix shuffle) as byte offsets and
    # reports false overlaps between distinct pool slots; the value
    # semantics were validated against hardware.
    nc = bacc.Bacc(
        "TRN2", target_bir_lowering=False, debug=False,
        detect_race_conditions=False,
    )

    xT_d = nc.dram_tensor("xT", [128, KT * T2], BF16, kind="ExternalInput")
    wqk_d = nc.dram_tensor("wqk", [128, KT * 1536], BF16, kind="ExternalInput")
    wv_d = nc.dram_tensor("wv", [128, KT * 768], BF16, kind="ExternalInput")
    wp_d = nc.dram_tensor("wp", [128, KT * 768], BF16, kind="ExternalInput")
    relb_d = nc.dram_tensor("relb", [100, 2 * H * P2], BF16, kind="ExternalInput")
    mix_d = nc.dram_tensor("mixblk", [120, 120], BF16, kind="ExternalInput")
    bqk_d = nc.dram_tensor("bqk", [128, QKM], F32, kind="ExternalInput")
    bv_d = nc.dram_tensor("bv", [128, 768], BF16, kind="ExternalInput")
    bp_d = nc.dram_tensor("bp", [128, 768], BF16, kind="ExternalInput")
    out_d = nc.dram_tensor("out", [T2, 768], BF16, kind="ExternalOutput")

    with tile.TileContext(nc) as tc:
        with (
            tc.tile_pool(name="const", bufs=1) as cpool,
            tc.tile_pool(name="qkv", bufs=1) as qkvpool,
            tc.tile_pool(name="mxin", bufs=3) as mxpool,
            tc.tile_pool(name="dram", bufs=2, space=bass.MemorySpace.DRAM) as drpool,
            tc.tile_pool(name="dramA", bufs=BL, space=bass.MemorySpace.DRAM) as drApool,
            tc.tile_pool(name="psA", bufs=2, space=bass.MemorySpace.PSUM) as psA,
        ):
            # ---- constants ----
            wqk = cpool.tile([128, KT * 1536], BF16)
            wv = cpool.tile([128, KT * 768], BF16)
            wp = cpool.tile([128, KT * 768], BF16)
            relb = cpool.tile([100, 2 * H * P2], BF16)
            mixblk = cpool.tile([120, 120], BF16)
            bqk = cpool.tile([128, QKM], F32)
            bv = cpool.tile([128, 768], BF16)   # v-bias rows, replicated
            bp = cpool.tile([128, 768], BF16)   # proj-bias rows, replicated
            ones_den = cpool.tile([128, 128], BF16)
            nc.vector.memset(ones_den[:], 1.0)

            # persistent per-core activations. qk lives in 12 separate
            # tiles (one per 128-channel group) so attention ticks only
            # depend on their own stage-1 GEMMs.
            qk_t = [qkvpool.tile([128, T2], BF16, name=f"qk{mt}")
                    for mt in range(QKM)]
            v_sb = qkvpool.tile([100, 2 * BL * 768], BF16)
            scrA = [drApool.tile([10, 10, H, 2 * P2], BF16, tag="scrA",
                                 name=f"scrA{b}")
                    for b in range(BL)]

            with (
                tc.tile_pool(name="xin", bufs=1) as xpool,
                tc.tile_pool(name="et", bufs=4) as etpool,
                tc.tile_pool(name="dch", bufs=3) as dchpool,
                tc.tile_pool(name="psS", bufs=3, space=bass.MemorySpace.PSUM) as psS,
                tc.tile_pool(name="psD", bufs=2, space=bass.MemorySpace.PSUM) as psD,
            ):
                xT = xpool.tile([128, KT * T2], BF16)
                for kt in range(KT):
                    nc.sync.dma_start(
                        xT[:, kt * T2 : (kt + 1) * T2],
                        xT_d[:, kt * T2 : (kt + 1) * T2],
                    )
                    nc.sync.dma_start(
                        wqk[:, kt * 1536 : (kt + 1) * 1536],
                        wqk_d[:, kt * 1536 : (kt + 1) * 1536],
                    )
                nc.sync.dma_start(relb[:], relb_d[:])
                nc.sync.dma_start(bqk[:], bqk_d[:])
                nc.sync.dma_start(wv[:], wv_d[:])
                nc.sync.dma_start(bv[:], bv_d[:])
                nc.sync.dma_start(mixblk[:], mix_d[:])
                nc.sync.dma_start(wp[:], wp_d[:])
                nc.sync.dma_start(bp[:], bp_d[:])

                def qk_gemm(mt):
                    for (n0, nsz) in TOK_CHUNKS:
                        ps = psA.tile([128, 512], F32, tag="a")
                        for kt in range(KT):
                            nc.tensor.matmul(
                                ps[:, 0:nsz],
                                wqk[:, kt * 1536 + mt * 128 : kt * 1536 + (mt + 1) * 128],
                                xT[:, kt * T2 + n0 : kt * T2 + n0 + nsz],
                                start=(kt == 0),
                                stop=(kt == KT - 1),
                            )
                        nc.scalar.activation(
                            qk_t[mt][:, n0 : n0 + nsz],
                            ps[:, 0:nsz],
                            AF.Identity,
                            bias=bqk[:, mt : mt + 1],
                            scale=1.0,
                        )

                def a_chunk(b, tq):
                    """scores/exp/relb/den/normalize for heads 4tq..4tq+3 of
                    batch b, dumped to scrA[b][:, :, 4tq:4tq+4, :]."""
                    et = etpool.tile([100, 4 * 2 * P2], BF16, tag="et")
                    etv = et[:].rearrange("p (h c n) -> p h c n", h=4, c=2, n=P2)
                    for hh in range(4):
                        h = 4 * tq + hh
                        prow = (h % 2) * 64
                        qt = qk_t[h // 2]
                        kt_ = qk_t[6 + h // 2]
                        ps1 = psS.tile([128, 512], F32, tag="s")
                        nc.tensor.matmul(
                            ps1[0:100, 0:P2],
                            kt_[prow : prow + 64, b * P2 : b * P2 + 100],
                            qt[prow : prow + 64, b * P2 : b * P2 + P2],
                            start=True, stop=True,
                        )
                        nc.tensor.matmul(
                            ps1[0:100, P2 : 2 * P2],
                            kt_[prow : prow + 64, b * P2 + 100 : b * P2 + 200],
                            qt[prow : prow + 64, b * P2 : b * P2 + P2],
                            start=True, stop=True,
                        )
                        nc.scalar.activation(
                            et[0:100, hh * 2 * P2 : (hh + 1) * 2 * P2],
                            ps1[0:100, 0 : 2 * P2], AF.Exp,
                        )
                    # numerator: exp(scores) * exp(bias)
                    nc.vector.tensor_tensor(
                        et[0:100, :], et[0:100, :],
                        relb[0:100, 4 * tq * 2 * P2 : (4 * tq + 4) * 2 * P2],
                        ALU.mult,
                    )
                    # denominators (replicated across partitions by the PE)
                    # and in-place normalize
                    dch = dchpool.tile([100, 800], F32, tag="dch")
                    for dd in range(2):
                        psd = psD.tile([128, 512], F32, tag="d")
                        nc.tensor.matmul(
                            psd[0:100, 0:400],
                            ones_den[0:100, 0:100],
                            etv[:, 2 * dd : 2 * dd + 2, 0, :],
                            start=True, stop=False,
                        )
                        nc.tensor.matmul(
                            psd[0:100, 0:400],
                            ones_den[0:100, 0:100],
                            etv[:, 2 * dd : 2 * dd + 2, 1, :],
                            start=False, stop=True,
                        )
                        nc.vector.reciprocal_approx_fast(
                            dch[:, dd * 400 : (dd + 1) * 400],
                            psd[0:100, 0:400],
                        )
                    dv4 = dch[:].rearrange("p (h n) -> p h n", h=4)
                    for c in range(2):
                        nc.vector.tensor_tensor(
                            etv[:, :, c, :], etv[:, :, c, :], dv4, ALU.mult
                        )
                    nc.sync.dma_start(
                        scrA[b][:].rearrange(
                            "j wgi h cn -> (j wgi) h cn"
                        )[:, 4 * tq : 4 * tq + 4, :],
                        et[0:100, :].rearrange("p (h cn) -> p h cn", cn=2 * P2),
                    )

                # ---- interleaved stage 1 + attention phase A ----
                for tq in range(3):
                    for mt in (2 * tq, 2 * tq + 1, 6 + 2 * tq, 6 + 2 * tq + 1):
                        qk_gemm(mt)
                    for b in range(BL):
                        a_chunk(b, tq)

                # early B-phase gathers overlap the v projection below
                def hop2(b):
                    mxin = mxpool.tile([120, 10 * 2 * P2], BF16, tag="mxin")
                    nc.sync.dma_start(
                        mxin[:].rearrange("r (j cn) -> r j cn", j=10),
                        scrA[b][:].rearrange("j wgi h cn -> (wgi h) j cn"),
                    )
                    return mxin

                mx0 = hop2(0)
                mx1 = hop2(1)

                # ---- v = xT.T @ Wv', bias added in the Vector drain ----
                for b in range(BL):
                    for c in range(2):
                        base = b * P2 + c * 100
                        for (n0, nsz) in [(0, 512), (512, 256)]:
                            ps = psA.tile([128, 512], F32, tag="a")
                            for kt in range(KT):
                                nc.tensor.matmul(
                                    ps[0:100, 0:nsz],
                                    xT[:, kt * T2 + base : kt * T2 + base + 100],
                                    wv[:, kt * 768 + n0 : kt * 768 + n0 + nsz],
                                    start=(kt == 0),
                                    stop=(kt == KT - 1),
                                )
                            nc.vector.tensor_tensor(
                                v_sb[0:100, (b * 2 + c) * 768 + n0 : (b * 2 + c) * 768 + n0 + nsz],
                                ps[0:100, 0:nsz],
                                bv[0:100, n0 : n0 + nsz],
                                ALU.add,
                            )

            # xT/et/dch + psS freed; phase-B SBUF pools reuse the space.
            with (
                tc.tile_pool(name="mxout", bufs=2) as mopool,
                tc.tile_pool(name="a2", bufs=2) as a2pool,
                tc.tile_pool(name="ao", bufs=2) as aopool,
                tc.tile_pool(name="osb", bufs=2) as opool,
                tc.tile_pool(name="psM", bufs=3, space=bass.MemorySpace.PSUM) as psM,
                tc.tile_pool(name="psV", bufs=2, space=bass.MemorySpace.PSUM) as psV,
            ):
                def b_mix(b, mxin):
                    """mix matmul -> shuffle back (hop3 + hop4)."""
                    mxo = mopool.tile([120, 10 * 2 * P2], BF16, tag="mxo")
                    for i, o in enumerate(range(0, 4000, 500)):
                        psm = psM.tile([128, 512], F32, tag="m")
                        nc.tensor.matmul(
                            psm[0:120, 0:500], mixblk[:],
                            mxin[:, o : o + 500],
                            start=True, stop=True,
                        )
                        nc.vector.tensor_scalar_add(
                            mxo[:, o : o + 500], psm[0:120, 0:500], 0.0
                        )

                    # scrB byte order (j, wgi, k, cn): hop3 scatters 800B
                    # runs; hop4 is a fully contiguous gather into a2.
                    scrB = drpool.tile([10, 10, H, 2 * P2], BF16, tag="scrB")
                    nc.sync.dma_start(
                        scrB[:].rearrange("j wgi k cn -> (wgi k) j cn"),
                        mxo[:].rearrange("r (j cn) -> r j cn", j=10),
                    )
                    a2 = a2pool.tile([100, 2 * H * P2], BF16, tag="a2")
                    nc.sync.dma_start(
                        a2[0:100, :],
                        scrB[:].rearrange("j wgi k cn -> (j wgi) (k cn)"),
                    )
                    return a2

                def b_avproj(b, a2):
                    """AV (head pairs share a psum via tile_position) and
                    the output projection (bias added in the Vector drain)."""
                    aoT = aopool.tile([128, KT * P2], BF16, tag="ao")
                    for jj in range(H // 2):
                        pv = psV.tile([128, 512], F32, tag="v")
                        for sub in range(2):
                            k = 2 * jj + sub
                            rows = pv[sub * 64 : sub * 64 + 64, 0:P2]
                            tp = (0, sub * 64)
                            for c in range(2):
                                nc.tensor.matmul(
                                    rows,
                                    v_sb[0:100, (b * 2 + c) * 768 + k * 64 : (b * 2 + c) * 768 + (k + 1) * 64],
                                    a2[0:100, k * 2 * P2 + c * P2 : k * 2 * P2 + c * P2 + P2],
                                    start=(c == 0),
                                    stop=(c == 1),
                                    tile_position=tp,
                                )
                        nc.scalar.copy(
                            aoT[:, jj * P2 : (jj + 1) * P2], pv[:, 0:P2]
                        )

                    for (t0, tsz) in [(0, 128), (128, 72)]:
                        osb = opool.tile([128, 768], BF16, tag="osb")
                        for (n0, nsz) in [(0, 512), (512, 256)]:
                            pp = psA.tile([128, 512], F32, tag="a")
                            for kt in range(KT):
                                nc.tensor.matmul(
                                    pp[0:tsz, 0:nsz],
                                    aoT[:, kt * P2 + t0 : kt * P2 + t0 + tsz],
                                    wp[:, kt * 768 + n0 : kt * 768 + n0 + nsz],
                                    start=(kt == 0),
                                    stop=(kt == KT - 1),
                                )
                            nc.vector.tensor_tensor(
                                osb[0:tsz, n0 : n0 + nsz],
                                pp[0:tsz, 0:nsz],
                                bp[0:tsz, n0 : n0 + nsz],
                                ALU.add,
                            )
                        nc.sync.dma_start(
                            out_d[b * P2 + t0 : b * P2 + t0 + tsz, :], osb[0:tsz, :]
                        )

                # software pipeline: mix/shuffle of batch b+1 is emitted
                # ahead of AV/projection of batch b.
                mx = {0: mx0, 1: mx1}
                mx[2] = hop2(2)
                a2s = {0: b_mix(0, mx.pop(0))}
                for b in range(BL):
                    if b + 3 < BL:
                        mx[b + 3] = hop2(b + 3)
                    if b + 1 < BL:
                        a2s[b + 1] = b_mix(b + 1, mx.pop(b + 1))
                    b_avproj(b, a2s.pop(b))

    nc.compile()
    return nc


def _tile6(a, width):
    """[768, M] -> [128, 6*M] (K-tile-major host layout)."""
    assert a.shape == (768, width)
    return np.ascontiguousarray(
        a.reshape(KT, 128, width).transpose(1, 0, 2).reshape(128, KT * width)
    )


def _to_bf16(a):
    return np.asarray(a, dtype=np.float32).astype(ml_dtypes.bfloat16)


def _posmaps():
    """token m -> padded position p, and p -> m (or -1 for dummies)."""
    pos_of_tok = np.empty(N, np.int64)
    for m in range(N):
        c = 0 if m < 100 else 1
        mm = m - c * 100
        g, ml = mm // 10, mm % 10
        pos_of_tok[m] = c * 100 + ml * 10 + g
    tok_of_pos = np.full(P2, -1, np.int64)
    tok_of_pos[pos_of_tok] = np.arange(N)
    return pos_of_tok, tok_of_pos


_POS_OF_TOK, _TOK_OF_POS = _posmaps()


def _preprocess(inputs):
    x = np.asarray(inputs["x"], np.float32)
    qkv_w = np.asarray(inputs["qkv_w"], np.float32)
    q_bias = np.asarray(inputs["q_bias"], np.float32)
    v_bias = np.asarray(inputs["v_bias"], np.float32)
    sq = np.asarray(inputs["ssf_scale_qkv"], np.float32)
    tq = np.asarray(inputs["ssf_shift_qkv"], np.float32)
    rbt = np.asarray(inputs["rel_bias_table"], np.float32)
    coeff = np.asarray(inputs["bases_coeff"], np.float32)
    proj_w = np.asarray(inputs["proj_w"], np.float32)
    proj_b = np.asarray(inputs["proj_b"], np.float32)
    sp = np.asarray(inputs["ssf_scale_proj"], np.float32)
    tp = np.asarray(inputs["ssf_shift_proj"], np.float32)
    rel_index = np.asarray(inputs["rel_index"], np.int64)

    qkv_bias = np.concatenate([q_bias, np.zeros_like(q_bias), v_bias])
    w_eff = (qkv_w * sq[:, None]).copy()
    b_eff = (qkv_bias * sq + tq).copy()
    w_eff[0:768] *= SCALE
    b_eff[0:768] *= SCALE

    wqk = _tile6(np.ascontiguousarray(w_eff[0:1536].T), 1536)
    wvt = _tile6(np.ascontiguousarray(w_eff[1536:].T), 768)
    wp_eff = proj_w * sp[:, None]
    bp_eff = proj_b * sp + tp
    wpt = _tile6(np.ascontiguousarray(wp_eff.T), 768)

    bqk_sb = np.ascontiguousarray(b_eff[0:1536].reshape(QKM, 128).T).astype(np.float32)

    # rel bias in permuted+padded coordinates:
    # relb[p, (h*2+c)*P2 + n] = table[rel_index[qtok(n), ktok(c,p)], h]
    # dummy keys get DUMMY_BIAS, dummy queries 0.
    gathered = rbt[rel_index]                      # [query-tok, key-tok, H]
    relb4 = np.zeros((100, H, 2, P2), np.float32)
    q_valid = _TOK_OF_POS >= 0                     # [P2]
    qtok = np.where(q_valid, _TOK_OF_POS, 0)
    for c in range(2):
        ktok_pos = _TOK_OF_POS[c * 100 : (c + 1) * 100]   # [100]
        k_valid = ktok_pos >= 0
        ktok = np.where(k_valid, ktok_pos, 0)
        # blk[p, h, n] = gathered[qtok[n], ktok[p], h]
        blk = gathered[qtok[None, :], ktok[:, None], :]   # [100, P2, H]
        blk = blk.transpose(0, 2, 1)                      # [100, H, P2]
        blk = np.where(q_valid[None, None, :], blk, 0.0)
        blk = np.where(k_valid[:, None, None], blk, DUMMY_BIAS)
        relb4[:, :, c, :] = blk
    # upload exp(bias): the kernel multiplies exp(scores) by this instead
    # of adding the bias before the exp (dummy keys -> exp(-40) ~ 0).
    relb = np.exp(relb4.reshape(100, 2 * H * P2))

    # mix = coeff^T * 1.0 + I ; mixblk[wgi*12+h, wgi'*12+k] = d(wgi,wgi')mix[h,k]
    mix = coeff.T + np.eye(H, dtype=np.float32)
    mixblk = np.kron(np.eye(10, dtype=np.float32), mix)
    bv_rep = np.broadcast_to(b_eff[1536:].reshape(1, 768), (128, 768))
    bp_rep = np.broadcast_to(bp_eff.reshape(1, 768), (128, 768))

    common = {
        "wqk": _to_bf16(wqk),
        "wv": _to_bf16(wvt),
        "wp": _to_bf16(wpt),
        "relb": _to_bf16(relb),
        "mixblk": _to_bf16(mixblk),
        "bqk": bqk_sb,
        "bv": _to_bf16(bv_rep),
        "bp": _to_bf16(bp_rep),
    }
    in_maps = []
    for ci in range(NCORES):
        xs = x[ci * BL : (ci + 1) * BL]             # [BL, N, C]
        xp = np.zeros((BL, P2, C), np.float32)
        xp[:, _POS_OF_TOK, :] = xs
        xt = xp.reshape(BL * P2, C).T               # [C, T2]
        m = dict(common)
        m["xT"] = _to_bf16(_tile6(np.ascontiguousarray(xt), T2))
        in_maps.append(m)
    return in_maps


def _get_compiled():
    if "nc" not in _COMPILED:
        _COMPILED["nc"] = _build_graph()
    return _COMPILED["nc"]


LAST_EXEC_NS = None
LAST_RESULTS = None


def _ensure_ntff_hook():
    """The agent image's antenv package lacks axon_hooks; synthesize it so
    run_bass_kernel_spmd(trace=True) can capture NTFF profiles."""
    import types

    if "antenv.axon_hooks" in sys.modules:
        return
    try:
        sys.path.insert(0, "/root/.axon_site")
        from trn_agent_boot.trn_boot import _ntff_profile_via_ctypes

        hook = _ntff_profile_via_ctypes("/opt/axon/libaxon_pjrt.so")
    except Exception:
        hook = None
    mod = types.ModuleType("antenv.axon_hooks")
    _state = {"hook": hook}
    mod.get_axon_ntff_profile_hook = lambda: _state["hook"]
    mod.set_axon_ntff_profile_hook = lambda h: _state.__setitem__("hook", h)
    sys.modules["antenv.axon_hooks"] = mod


def kernel(**inputs) -> np.ndarray:
    global LAST_EXEC_NS, LAST_RESULTS
    nc = _get_compiled()
    in_maps = _preprocess(inputs)
    from concourse.bass_utils import run_bass_kernel_spmd

    trace = os.environ.get("BASS_KERNEL_PROFILE", "0") == "1"
    if trace:
        _ensure_ntff_hook()
    res = run_bass_kernel_spmd(nc, in_maps, core_ids=list(range(NCORES)), trace=trace)
    LAST_EXEC_NS = res.exec_time_ns
    LAST_RESULTS = res
    outs = []
    for i in range(NCORES):
        o = np.asarray(res.results[i]["out"], dtype=np.float32).reshape(BL, P2, C)
        outs.append(o[:, _POS_OF_TOK, :])           # drop dummies, un-permute
    return np.concatenate(outs, axis=0).astype(np.float32)
